# revision 2
# baseline (speedup 1.0000x reference)
"""Local-strided block-sparse paged attention (decode) on 8 Trainium2 cores.

Strategy (v2):
- Work unit = (sequence b, kv-head kv). The 4 q-heads sharing a kv head
  attend overlapping block sets (shared local window + per-phase stride
  columns), so each unit loads the UNION of its 4 heads' CSR rows once
  and computes all 4 heads against it (per-head additive masks restore
  row membership + causality). This dedups ~2.5x of the K/V traffic and
  skips CSR padding entirely.
- Panels are converted to bf16 on host (tolerance is 2e-2; bf16 lands
  ~3e-3), halving HBM traffic again.
- 64 units are sorted by size and dealt round-robin to 8 cores so every
  core gets 8 size-matched slots; one SPMD program (sizes baked per
  slot) serves all cores. The program is recompiled if the size
  signature changes (inputs are resolved on host every call).
- Device per slot: chunked QK matmuls (scores[tok, 4 heads] in PSUM),
  masked exp -> P (bf16), PV matmuls against V panels carrying an extra
  ones-column so the softmax denominator falls out of the same
  accumulation; normalize and scatter to the output tile.
"""
import math
import numpy as np
import ml_dtypes

NCORES = 8
NEG = np.float32(-1e9)
_PROG_CACHE: dict = {}
_LAST_EXEC_NS = None


def _resolve_rows(layout_crow, layout_col, pbid, H, J=64):
    """Mirror the reference CSR row resolution (first-J trim + idx clip)."""
    W = layout_col.shape[1]
    rows = []
    for h in range(H):
        s = int(layout_crow[h, pbid])
        e = int(layout_crow[h, pbid + 1])
        n = min(max(e - s, 0), J)
        idx = np.clip(np.arange(s, s + n), 0, W - 1)
        rows.append(layout_col[h, idx].tolist())
    return rows


def _prepare(q, k_cache, v_cache, block_tables, context_lens, layout_crow, layout_col):
    B, H, D = q.shape
    KVH = k_cache.shape[1]
    BLK = v_cache.shape[3]
    X = k_cache.shape[4]
    G = H // KVH
    q_pid = context_lens.astype(np.int64) - 1
    pbid = q_pid // BLK

    bf16 = ml_dtypes.bfloat16

    # ---- build units: (b, kv) -> union block list + per-head validity ----
    units = []
    for b in range(B):
        rows_all = _resolve_rows(layout_crow, layout_col, int(pbid[b]), H)
        for kv in range(KVH):
            heads = [kv * G + j for j in range(G)]
            cnts = []
            for h in heads:
                c = {}
                for kb in rows_all[h]:
                    c[kb] = c.get(kb, 0) + 1
                cnts.append(c)
            mult = {}
            for c in cnts:
                for kb, n in c.items():
                    mult[kb] = max(mult.get(kb, 0), n)
            ulist = []
            copyidx = []
            for kb in sorted(mult):
                for i in range(mult[kb]):
                    ulist.append(kb)
                    copyidx.append(i)
            U = len(ulist)
            C = max(1, -(-U * BLK // 128))
            units.append(dict(b=b, kv=kv, heads=heads, ulist=ulist,
                              copyidx=copyidx, cnts=cnts, U=U, C=C))

    # ---- deal units to 8 cores x nslots slots, size-matched per slot ----
    nslots = -(-len(units) // NCORES)
    order = sorted(range(len(units)), key=lambda i: -units[i]["C"])
    slot_chunks = []
    assign = [[None] * nslots for _ in range(NCORES)]
    for k in range(nslots):
        grp = order[k * NCORES:(k + 1) * NCORES]
        slot_chunks.append(max(units[i]["C"] for i in grp))
        for c, i in enumerate(grp):
            assign[c][k] = i
    sig = tuple(slot_chunks) + (G, D)

    # ---- build per-core panels ----
    in_maps = []
    for c in range(NCORES):
        m = {}
        qq = np.zeros((D, G * nslots), bf16)
        for k in range(nslots):
            Ck = slot_chunks[k]
            T = Ck * 128
            kd = np.zeros((D, T), bf16)
            vt = np.zeros((128, Ck * 129), bf16)
            mt = np.zeros((128, Ck * G), bf16)
            ui = assign[c][k]
            if ui is not None:
                u = units[ui]
                b, kv, U = u["b"], u["kv"], u["U"]
                phys = block_tables[b, np.asarray(u["ulist"], np.int64)]
                # K: [U, D//X, BLK, X] -> [d = dx*X+xi, u*BLK+tok]
                kb = k_cache[phys, kv]
                kd[:, :U * BLK] = (
                    kb.transpose(1, 3, 0, 2).reshape(D, U * BLK).astype(bf16)
                )
                # V: [U, D, BLK] -> token-major [U*BLK, D], chunked + ones col
                vtok = np.zeros((T, D), np.float32)
                vtok[:U * BLK] = v_cache[phys, kv].transpose(0, 2, 1).reshape(U * BLK, D)
                varr = np.concatenate(
                    [vtok.reshape(Ck, 128, D),
                     np.ones((Ck, 128, 1), np.float32)], axis=2)
                vt[:] = varr.transpose(1, 0, 2).reshape(128, Ck * 129).astype(bf16)
                # mask [tok, head] -> [128, (chunk, head)]
                ul = np.asarray(u["ulist"], np.int64)
                ci = np.asarray(u["copyidx"], np.int64)
                pos = (ul[:, None] * BLK + np.arange(BLK)[None, :]).reshape(-1)
                mtok = np.zeros((T, G), np.float32)
                for j in range(G):
                    cnt = u["cnts"][j]
                    member = np.asarray(
                        [ci[i] < cnt.get(int(ul[i]), 0) for i in range(U)], bool)
                    ok = np.repeat(member, BLK) & (pos <= int(q_pid[b]))
                    mtok[:U * BLK, j] = ok.astype(np.float32)
                mt[:] = mtok.reshape(Ck, 128, G).transpose(1, 0, 2).reshape(
                    128, Ck * G).astype(bf16)
                qq[:, k * G:(k + 1) * G] = q[b, u["heads"]].T.astype(bf16)
            else:
                mt[:] = 1.0
            m[f"kd{k}"] = kd
            m[f"vm{k}"] = np.concatenate([vt, mt], axis=1)
        m["qq"] = qq
        # one mega K panel and one mega V+mask panel per core
        m["kg"] = np.concatenate(
            [m.pop(f"kd{k}") for k in range(nslots)], axis=1)
        m["vg"] = np.concatenate(
            [m.pop(f"vm{k}") for k in range(nslots)], axis=1)
        in_maps.append(m)
    return in_maps, assign, units, sig, nslots


def _build_program(sig, repeat=1, loop=0):
    import contextlib
    import concourse.bacc as bacc
    import concourse.mybir as mybir
    from concourse.tile import TileContext

    slot_chunks = list(sig[:-2])
    G, D = sig[-2], sig[-1]
    nslots = len(slot_chunks)
    f32 = mybir.dt.float32
    bf16 = mybir.dt.bfloat16
    SM = 1.0 / math.sqrt(D)

    nc = bacc.Bacc("TRN2", target_bir_lowering=False)
    tot = sum(slot_chunks)
    kg = nc.dram_tensor("kg", [D, tot * 128], bf16, kind="ExternalInput")
    vg = nc.dram_tensor("vg", [128, tot * (129 + G)], bf16,
                        kind="ExternalInput")
    qq = nc.dram_tensor("qq", [D, G * nslots], bf16, kind="ExternalInput")
    out = nc.dram_tensor("out", [G, nslots * D], f32, kind="ExternalOutput")

    with TileContext(nc) as tc:
        with (
            tc.tile_pool(name="kv", bufs=2) as kvp,
            tc.tile_pool(name="small", bufs=4) as sp,
            tc.tile_pool(name="ps_sc", bufs=4, space="PSUM") as pp_sc,
            tc.tile_pool(name="ps_ov", bufs=4, space="PSUM") as pp_ov,
            tc.tile_pool(name="persist", bufs=2) as cp,
        ):
            def _one_body():
                qt = cp.tile([D, G * nslots], bf16, tag="qt")
                nc.sync.dma_start(out=qt[:], in_=qq[:])
                kgt = kvp.tile([D, tot * 128], bf16, tag="kg")
                nc.sync.dma_start(out=kgt[:], in_=kg[:])
                vgt = kvp.tile([128, tot * (129 + G)], bf16, tag="vg")
                nc.scalar.dma_start(out=vgt[:], in_=vg[:])
                osb = cp.tile([G, nslots * D], f32, tag="osb")
                koff = 0
                voff = 0
                if True:
                  for k in range(nslots):
                    Ck = slot_chunks[k]
                    kt = kgt[:, koff:koff + Ck * 128]
                    koff += Ck * 128
                    vmt = vgt[:, voff:voff + Ck * (129 + G)]
                    voff += Ck * (129 + G)
                    vt = vmt[:, 0:Ck * 129]
                    mt = vmt[:, Ck * 129:Ck * (129 + G)]

                    sc = pp_sc.tile([128, Ck * G], f32, tag="sc")
                    for c in range(Ck):
                        nc.tensor.matmul(
                            sc[:, c * G:(c + 1) * G],
                            kt[:, c * 128:(c + 1) * 128],
                            qt[:, k * G:(k + 1) * G],
                            start=True, stop=True,
                        )
                    pe = sp.tile([128, Ck * G], bf16, tag="pe")
                    nc.scalar.activation(
                        pe[:], sc[:], mybir.ActivationFunctionType.Exp, scale=SM)
                    p = sp.tile([128, Ck * G], bf16, tag="p")
                    nc.vector.tensor_mul(p[:], pe[:], mt[:])
                    ov = pp_ov.tile([G, 129], f32, tag="ov")
                    for c in range(Ck):
                        nc.tensor.matmul(
                            ov[:], p[:, c * G:(c + 1) * G],
                            vt[:, c * 129:(c + 1) * 129],
                            start=(c == 0), stop=(c == Ck - 1),
                        )
                    rec = sp.tile([G, 1], f32, tag="rec")
                    nc.vector.reciprocal(rec[:], ov[:, 128:129])
                    nc.vector.tensor_scalar_mul(
                        osb[:, k * D:(k + 1) * D], ov[:, 0:128], rec[:])
                  nc.sync.dma_start(out=out[:], in_=osb[:])

            if loop:
                with tc.For_i(0, loop, 1,
                              hint_engines=(mybir.EngineType.PE,
                                            mybir.EngineType.DVE,
                                            mybir.EngineType.Activation)):
                    for _rep in range(repeat):
                        _one_body()
            else:
                for _rep in range(repeat):
                    _one_body()
    nc.compile()
    return nc


def _get_program(sig, repeat=1, loop=0):
    key = (sig, repeat, loop)
    nc = _PROG_CACHE.get(key)
    if nc is None:
        nc = _build_program(sig, repeat, loop)
        _PROG_CACHE[key] = nc
    return nc


def kernel(q, k_cache, v_cache, block_tables, context_lens, layout_crow, layout_col):
    from concourse.bass_utils import run_bass_kernel_spmd

    q = np.asarray(q, np.float32)
    k_cache = np.asarray(k_cache, np.float32)
    v_cache = np.asarray(v_cache, np.float32)
    block_tables = np.asarray(block_tables, np.int64)
    context_lens = np.asarray(context_lens, np.int64)
    layout_crow = np.asarray(layout_crow, np.int64)
    layout_col = np.asarray(layout_col, np.int64)

    B, H, D = q.shape
    KVH = k_cache.shape[1]
    G = H // KVH

    in_maps, assign, units, sig, nslots = _prepare(
        q, k_cache, v_cache, block_tables, context_lens, layout_crow, layout_col)

    nc = _get_program(sig)

    res = run_bass_kernel_spmd(nc, in_maps, core_ids=list(range(NCORES)))

    out = np.empty((B, H, D), np.float32)
    for c in range(NCORES):
        o = res.results[c]["out"]
        for k in range(nslots):
            ui = assign[c][k]
            if ui is None:
                continue
            u = units[ui]
            out[u["b"], u["heads"]] = o[:, k * D:(k + 1) * D]
    return out
